# revision 16
# baseline (speedup 1.0000x reference)
"""Trainium2 Bass kernel for nn_AgentEmbedding (embedding_lookup).

Contract: kernel(**inputs) takes the FULL unsharded inputs (numpy arrays,
keyed as in setup_inputs()) and returns the FULL [64, 50, 128] float32
output. Internally the batch dim B=64 is sharded 8-ways (8 per core);
the small linear weights are algebraically fused on the host (the module
is linear end-to-end) and replicated.

Per-core device program (B_local=8, T=400 tokens, E=128):
  1. gather row ids = idx_float + b*10000, computed on DVE (int32)
  2. 7 indirect-DMA gathers (one row id per dest partition, 512B rows)
     pull the 2*T=800 needed rows out of the flat [80000, 128] table
  3. PE transposes gathered tiles to feature-major [E, tok]
  4. one fused linear: PSUM accumulation of 4 fp32r matmuls
     (features+bias via homogeneous ones-row, graph via broadcast rhs,
      two gathered-embedding terms)
  5. PE transposes back to token-major, DMA out
"""

import os
import numpy as np

B, M, N, E = 64, 50, 10000, 128
NCORES = 8
BL = B // NCORES            # batches per core
T = BL * M                  # tokens per core
R = 2 * T                   # gathered rows per core
NG = 7                      # ceil(800/128) gather instructions
CHUNKS = [(0, 128), (128, 128), (256, 128), (384, 16)]  # output chunks

_cache = {}

last_exec_time_ns = None


def _install_trace_shims():
    """antenv.axon_hooks is absent in this image; register the NTFF hook
    ourselves so run_bass_kernel_spmd(trace=True) works under axon."""
    import sys, types
    if "antenv.axon_hooks" not in sys.modules:
        mod = types.ModuleType("antenv.axon_hooks")
        store = {}
        mod.set_axon_ntff_profile_hook = lambda h: store.__setitem__("h", h)
        mod.get_axon_ntff_profile_hook = lambda: store.get("h")
        sys.modules["antenv.axon_hooks"] = mod
        try:
            from trn_agent_boot.trn_boot import _ntff_profile_via_ctypes
            mod.set_axon_ntff_profile_hook(
                _ntff_profile_via_ctypes("/opt/axon/libaxon_pjrt.so")
            )
        except Exception:
            pass
    import concourse.bass_utils as bu
    bu.upload_artifacts = lambda d: d  # zero-egress container


def _gather_plan():
    """For each gather chunk j: (rows, [(k, tok0, col0, width), ...])
    mapping transpose output columns to g{k}T column ranges."""
    plan = []
    for j in range(NG):
        r0 = j * 128
        cnt = min(128, R - r0)
        segs = []
        r = r0
        while r < r0 + cnt:
            k, t = divmod(r, T)
            width = min(r0 + cnt - r, T - t)
            segs.append((k, t, r - r0, width))
            r += width
        plan.append((cnt, segs))
    return plan


def _build_nc():
    """Build + compile the per-core Bass program (SPMD: same program on
    all 8 cores, per-core input data)."""
    import concourse.bass as bass
    import concourse.bacc as bacc
    import concourse.mybir as mybir
    import concourse.tile as tile
    from concourse.masks import make_identity

    f32 = mybir.dt.float32
    f32r = mybir.dt.float32r
    i32 = mybir.dt.int32

    nc = bacc.Bacc("TRN2", target_bir_lowering=False)
    with tile.TileContext(nc) as tc:
        with tc.tile_pool(name="dram", bufs=1, space="DRAM") as dram:
            cities = dram.tile([BL * N, E], f32, kind="ExternalInput", name="cities")
            idx2 = dram.tile([128, 16], f32, kind="ExternalInput", name="idx2")
            featw = dram.tile([12, 528], f32, kind="ExternalInput", name="featw")
            wbig = dram.tile([128, 384], f32, kind="ExternalInput", name="wbig")
            graphT = dram.tile([128, 8], f32, kind="ExternalInput", name="graphT")
            out = dram.tile([T, E], f32, kind="ExternalOutput", name="out")
            names = dict(cities=cities.name, idx2=idx2.name, featw=featw.name,
                         wbig=wbig.name, graphT=graphT.name, out=out.name)

            with (
                tc.tile_pool(name="sb", bufs=1) as sb,
                tc.tile_pool(name="sbg", bufs=4) as sbg,
                tc.tile_pool(name="sbo", bufs=4) as sbo,
                tc.tile_pool(name="sba", bufs=4) as sba,
                tc.tile_pool(name="psA", bufs=4, space="PSUM") as psA,
                tc.tile_pool(name="psB", bufs=1, space="PSUM") as psB,
                tc.tile_pool(name="psC", bufs=1, space="PSUM") as psC,
                tc.tile_pool(name="psD", bufs=2, space="PSUM") as psD,
            ):
                idx2_sb = sb.tile([128, 16], f32, name="idx2_sb")
                nc.sync.dma_start(out=idx2_sb[:], in_=idx2[:])
                graphT_sb = sb.tile([128, 8], f32, name="graphT_sb")
                nc.sync.dma_start(out=graphT_sb[:], in_=graphT[:])
                featw_sb = sb.tile([12, 528], f32, name="featw_sb")
                nc.scalar.dma_start(out=featw_sb[:], in_=featw[:])
                wbig_sb = sb.tile([128, 384], f32, name="wbig_sb")
                nc.scalar.dma_start(out=wbig_sb[:], in_=wbig[:])

                ident = sb.tile([128, 128], f32, name="ident")
                make_identity(nc, ident[:])

                # idx = idx_float + b*10000; exact-integer floats, so the
                # f32 -> i32 conversion is exact under any rounding mode
                idxi = sb.tile([128, 8], i32, name="idxi")
                nc.vector.tensor_tensor(
                    out=idxi[:], in0=idx2_sb[:, 0:8], in1=idx2_sb[:, 8:16],
                    op=mybir.AluOpType.add)

                # fp32r operands must be produced by a rounding instruction
                featw_r = sb.tile([12, 528], f32r, name="featw_r")
                nc.vector.tensor_copy(out=featw_r[:], in_=featw_sb[:])
                wbig_r = sb.tile([128, 384], f32r, name="wbig_r")
                nc.vector.tensor_copy(out=wbig_r[:], in_=wbig_sb[:])
                graphT_r = sb.tile([128, 8], f32r, name="graphT_r")
                nc.vector.tensor_copy(out=graphT_r[:], in_=graphT_sb[:])

                # separate per-(k, output-chunk) tiles so chunk matmuls
                # only depend on the gathers that feed them
                gT = [[sb.tile([128, cc], f32r, name=f"g{k}T_{c}")
                       for c, (oo, cc) in enumerate(CHUNKS)]
                      for k in (0, 1)]
                for j, (cnt, segs) in enumerate(_gather_plan()):
                    ga = sbg.tile([128, E], f32, tag="ga", name=f"ga_{j}")
                    nc.gpsimd.indirect_dma_start(
                        out=ga[:cnt, :],
                        out_offset=None,
                        in_=cities[:, :],
                        in_offset=bass.IndirectOffsetOnAxis(
                            ap=idxi[:cnt, j:j + 1], axis=0),
                    )
                    pt = psA.tile([128, 128], f32, tag="pt", name=f"pt_{j}")
                    nc.tensor.transpose(
                        out=pt[:, :cnt], in_=ga[:cnt, :],
                        identity=ident[:cnt, :cnt])
                    for (k, t0, c0, w) in segs:
                        # split the segment at output-chunk boundaries
                        t = t0
                        while t < t0 + w:
                            c = min(t // 128, 3)
                            o_c = CHUNKS[c][0]
                            wp = min(t0 + w - t, CHUNKS[c][0] + CHUNKS[c][1] - t)
                            nc.vector.tensor_copy(
                                out=gT[k][c][:, t - o_c:t - o_c + wp],
                                in_=pt[:, c0 + (t - t0):c0 + (t - t0) + wp])
                            t += wp

                # feat + graph close their PSUM group early (no gather dep),
                # then get pre-transposed to token-major while gathers run
                po = psB.tile([128, T], f32, name="po")
                nc.tensor.matmul(out=po[:], lhsT=featw_r[:, 400:528],
                                 rhs=featw_r[:, 0:400],
                                 start=True, stop=False)
                nc.tensor.matmul(out=po[:], lhsT=wbig_r[:, 256:384],
                                 rhs=graphT_r[:, :].to_broadcast([128, BL, M]),
                                 start=False, stop=True)
                ctx_sb = sb.tile([128, T], f32, name="ctx_sb")
                nc.scalar.copy(out=ctx_sb[:], in_=po[:])
                ctxT = []
                for c, (o, cnt) in enumerate(CHUNKS):
                    pt2 = psC.tile([128, 128], f32, tag="pt2", name=f"pt2_{c}")
                    nc.tensor.transpose(
                        out=pt2[:cnt, :], in_=ctx_sb[:, o:o + cnt],
                        identity=ident[:, :])
                    ctxT_c = sba.tile([128, 128], f32, tag=f"ctxT_{c}",
                                      name=f"ctxT_{c}")
                    nc.scalar.copy(out=ctxT_c[:cnt, :], in_=pt2[:cnt, :])
                    ctxT.append(ctxT_c)

                # per-chunk gathered-embedding matmuls, token-major (the
                # gathered tile is lhsT), so each chunk's PSUM is already
                # in output layout: one DVE add then DMA out
                for c, (o, cnt) in enumerate(CHUNKS):
                    pg = psD.tile([128, 128], f32, tag="pg", name=f"pg_{c}")
                    nc.tensor.matmul(out=pg[:cnt, :], lhsT=gT[0][c][:, :],
                                     rhs=wbig_r[:, 0:128],
                                     start=True, stop=False)
                    nc.tensor.matmul(out=pg[:cnt, :], lhsT=gT[1][c][:, :],
                                     rhs=wbig_r[:, 128:256],
                                     start=False, stop=True)
                    ob = sbo.tile([128, E], f32, tag="ob", name=f"ob_{c}")
                    nc.vector.tensor_tensor(
                        out=ob[:cnt, :], in0=ctxT[c][:cnt, :],
                        in1=pg[:cnt, :], op=mybir.AluOpType.add)
                    eng = nc.sync if c % 2 == 0 else nc.scalar
                    eng.dma_start(out=out[o:o + cnt, :], in_=ob[:cnt, :])

    nc.compile()
    return nc, names


def _host_prep(inputs):
    """Fuse the linear layers (the module has no nonlinearity) and lay out
    per-core device inputs."""
    f64 = np.float64
    W_a = np.asarray(inputs["W_a"], f64)
    Wa0, Wa1 = W_a[:, :E], W_a[:, E:]
    W_dp = np.asarray(inputs["W_dp"], f64)
    Wf0 = Wa1 @ W_dp[:, :E]
    Wf1 = Wa1 @ W_dp[:, E:]
    Wfc = Wa1 @ np.asarray(inputs["W_dc"], f64)
    Wfn = Wa1 @ np.asarray(inputs["W_nc"], f64)
    Wfp = Wa1 @ np.asarray(inputs["W_ps"], f64)
    Wfg = Wa0 @ np.asarray(inputs["W_g"], f64)
    b_sum = (np.asarray(inputs["b_dp"], f64) + np.asarray(inputs["b_dc"], f64)
             + np.asarray(inputs["b_nc"], f64) + np.asarray(inputs["b_ps"], f64))
    b_total = (np.asarray(inputs["b_a"], f64) + Wa1 @ b_sum
               + Wa0 @ np.asarray(inputs["b_g"], f64))

    # featw: [12, 528] = [features+ones | (Wff_ext)^T]
    Wff_ext = np.concatenate(
        [Wfc, Wfn, Wfp, b_total[:, None]], axis=1)          # [128, 12]
    # wbig: [128, 384] = [Wf0^T | Wf1^T | Wfg^T]
    wbig = np.concatenate([Wf0.T, Wf1.T, Wfg.T], axis=1).astype(np.float32)
    wbig = np.ascontiguousarray(wbig)

    cities_embed = np.asarray(inputs["cities_embed"], np.float32)
    graph_embed = np.asarray(inputs["graph_embed"], np.float32)
    agent_state = np.asarray(inputs["agent_state"], np.float32)

    # packed gather grid: slot (p, j) is flat row r = j*128 + p, where
    # r = k*T + t; col j+8 carries the b*10000 offset constant
    p_grid = np.arange(128)[:, None]
    j_grid = np.arange(NG + 1)[None, :]          # 8 cols, col 7 unused pad
    r_grid = j_grid * 128 + p_grid
    valid = r_grid < R
    k_grid = np.where(valid, r_grid // T, 0)
    t_grid = np.where(valid, r_grid % T, 0)
    bofs = ((t_grid // M) * N * valid).astype(np.float32)

    in_maps = []
    for core in range(NCORES):
        bsl = slice(core * BL, (core + 1) * BL)
        ag = agent_state[bsl].reshape(T, 13)
        idx2 = np.zeros((128, 16), np.float32)
        idx2[:, 0:8] = ag[t_grid, k_grid] * valid
        idx2[:, 8:16] = bofs
        featw = np.zeros((12, 528), np.float32)
        featw[:11, :T] = ag[:, 2:13].T
        featw[11, :T] = 1.0
        featw[:, 400:528] = Wff_ext.T.astype(np.float32)
        in_maps.append({
            "cities": np.ascontiguousarray(cities_embed[bsl].reshape(BL * N, E)),
            "idx2": idx2,
            "featw": featw,
            "wbig": wbig,
            "graphT": np.ascontiguousarray(graph_embed[bsl, 0, :].T),
        })
    return in_maps


def kernel(**inputs):
    global last_exec_time_ns
    trace = os.environ.get("BASS_KERNEL_TRACE", "") == "1"
    if trace:
        _install_trace_shims()

    from concourse.bass_utils import run_bass_kernel_spmd

    if "nc" not in _cache:
        _cache["nc"], _cache["names"] = _build_nc()
    nc, names = _cache["nc"], _cache["names"]

    in_maps = []
    for m in _host_prep(inputs):
        in_maps.append({names[k]: v for k, v in m.items()})

    kwargs = {}
    if trace:
        tdir = os.environ.get("BASS_KERNEL_TRACE_DIR", "/tmp/kern_trace")
        import shutil
        shutil.rmtree(tdir, ignore_errors=True)
        os.makedirs(tdir, exist_ok=True)
        kwargs = dict(trace=True, trace_cores=list(range(NCORES)), tmpdir=tdir)
    res = run_bass_kernel_spmd(nc, in_maps, core_ids=list(range(NCORES)), **kwargs)
    last_exec_time_ns = res.exec_time_ns

    out = np.stack([res.results[i][names["out"]] for i in range(NCORES)])
    return out.reshape(B, M, E).astype(np.float32)


# revision 17
# speedup vs baseline: 1.1420x; 1.1420x over previous
"""Trainium2 Bass kernel for nn_AgentEmbedding (embedding_lookup).

Contract: kernel(**inputs) takes the FULL unsharded inputs (numpy arrays,
keyed as in setup_inputs()) and returns the FULL [64, 50, 128] float32
output. Internally the batch dim B=64 is sharded 8-ways (8 per core);
the small linear weights are algebraically fused on the host (the module
is linear end-to-end) and replicated.

Per-core device program (B_local=8, T=400 tokens, E=128):
  1. gather row ids = idx_float + b*10000, computed on DVE (int32)
  2. 7 indirect-DMA gathers (one row id per dest partition, 512B rows)
     pull the 2*T=800 needed rows out of the flat [80000, 128] table
  3. PE transposes gathered tiles to feature-major [E, tok]
  4. one fused linear: PSUM accumulation of 4 fp32r matmuls
     (features+bias via homogeneous ones-row, graph via broadcast rhs,
      two gathered-embedding terms)
  5. PE transposes back to token-major, DMA out
"""

import os
import numpy as np

B, M, N, E = 64, 50, 10000, 128
NCORES = 8
BL = B // NCORES            # batches per core
T = BL * M                  # tokens per core
R = 2 * T                   # gathered rows per core
NG = 7                      # ceil(800/128) gather instructions
CHUNKS = [(0, 128), (128, 128), (256, 128), (384, 16)]  # output chunks

_cache = {}

last_exec_time_ns = None


def _install_trace_shims():
    """antenv.axon_hooks is absent in this image; register the NTFF hook
    ourselves so run_bass_kernel_spmd(trace=True) works under axon."""
    import sys, types
    if "antenv.axon_hooks" not in sys.modules:
        mod = types.ModuleType("antenv.axon_hooks")
        store = {}
        mod.set_axon_ntff_profile_hook = lambda h: store.__setitem__("h", h)
        mod.get_axon_ntff_profile_hook = lambda: store.get("h")
        sys.modules["antenv.axon_hooks"] = mod
        try:
            from trn_agent_boot.trn_boot import _ntff_profile_via_ctypes
            mod.set_axon_ntff_profile_hook(
                _ntff_profile_via_ctypes("/opt/axon/libaxon_pjrt.so")
            )
        except Exception:
            pass
    import concourse.bass_utils as bu
    bu.upload_artifacts = lambda d: d  # zero-egress container


def _gather_plan():
    """For each gather chunk j: (rows, [(k, tok0, col0, width), ...])
    mapping transpose output columns to g{k}T column ranges."""
    plan = []
    for j in range(NG):
        r0 = j * 128
        cnt = min(128, R - r0)
        segs = []
        r = r0
        while r < r0 + cnt:
            k, t = divmod(r, T)
            width = min(r0 + cnt - r, T - t)
            segs.append((k, t, r - r0, width))
            r += width
        plan.append((cnt, segs))
    return plan


def _build_nc():
    """Build + compile the per-core Bass program (SPMD: same program on
    all 8 cores, per-core input data)."""
    import concourse.bass as bass
    import concourse.bacc as bacc
    import concourse.mybir as mybir
    import concourse.tile as tile
    from concourse.masks import make_identity

    f32 = mybir.dt.float32
    f32r = mybir.dt.float32r
    i32 = mybir.dt.int32

    nc = bacc.Bacc("TRN2", target_bir_lowering=False)
    with tile.TileContext(nc) as tc:
        with tc.tile_pool(name="dram", bufs=1, space="DRAM") as dram:
            cities = dram.tile([BL * N, E], f32, kind="ExternalInput", name="cities")
            idx2 = dram.tile([128, 16], f32, kind="ExternalInput", name="idx2")
            featw = dram.tile([12, 528], f32, kind="ExternalInput", name="featw")
            wbig = dram.tile([128, 384], f32, kind="ExternalInput", name="wbig")
            graphT = dram.tile([128, 8], f32, kind="ExternalInput", name="graphT")
            out = dram.tile([T, E], f32, kind="ExternalOutput", name="out")
            names = dict(cities=cities.name, idx2=idx2.name, featw=featw.name,
                         wbig=wbig.name, graphT=graphT.name, out=out.name)

            with (
                tc.tile_pool(name="sb", bufs=1) as sb,
                tc.tile_pool(name="sbg", bufs=4) as sbg,
                tc.tile_pool(name="sbo", bufs=4) as sbo,
                tc.tile_pool(name="sba", bufs=4) as sba,
                tc.tile_pool(name="psA", bufs=4, space="PSUM") as psA,
                tc.tile_pool(name="psB", bufs=1, space="PSUM") as psB,
                tc.tile_pool(name="psC", bufs=1, space="PSUM") as psC,
                tc.tile_pool(name="psD", bufs=2, space="PSUM") as psD,
            ):
                idx2_sb = sb.tile([128, 16], f32, name="idx2_sb")
                nc.sync.dma_start(out=idx2_sb[:], in_=idx2[:])
                graphT_sb = sb.tile([128, 8], f32, name="graphT_sb")
                nc.sync.dma_start(out=graphT_sb[:], in_=graphT[:])
                featw_sb = sb.tile([12, 528], f32, name="featw_sb")
                nc.scalar.dma_start(out=featw_sb[:], in_=featw[:])
                wbig_sb = sb.tile([128, 384], f32, name="wbig_sb")
                nc.scalar.dma_start(out=wbig_sb[:], in_=wbig[:])

                ident = sb.tile([128, 128], f32, name="ident")
                make_identity(nc, ident[:])

                # idx = idx_float + b*10000; exact-integer floats, so the
                # f32 -> i32 conversion is exact under any rounding mode
                idxi = sb.tile([128, 8], i32, name="idxi")
                nc.vector.tensor_tensor(
                    out=idxi[:], in0=idx2_sb[:, 0:8], in1=idx2_sb[:, 8:16],
                    op=mybir.AluOpType.add)

                # fp32r operands must be produced by a rounding instruction
                featw_r = sb.tile([12, 528], f32r, name="featw_r")
                nc.vector.tensor_copy(out=featw_r[:], in_=featw_sb[:])
                wbig_r = sb.tile([128, 384], f32r, name="wbig_r")
                nc.vector.tensor_copy(out=wbig_r[:], in_=wbig_sb[:])
                graphT_r = sb.tile([128, 8], f32r, name="graphT_r")
                nc.vector.tensor_copy(out=graphT_r[:], in_=graphT_sb[:])

                # separate per-(k, output-chunk) tiles so chunk matmuls
                # only depend on the gathers that feed them
                gT = [[sb.tile([128, cc], f32r, name=f"g{k}T_{c}")
                       for c, (oo, cc) in enumerate(CHUNKS)]
                      for k in (0, 1)]
                for j, (cnt, segs) in enumerate(_gather_plan()):
                    ga = sbg.tile([128, E], f32, tag="ga", name=f"ga_{j}")
                    nc.gpsimd.indirect_dma_start(
                        out=ga[:cnt, :],
                        out_offset=None,
                        in_=cities[:, :],
                        in_offset=bass.IndirectOffsetOnAxis(
                            ap=idxi[:cnt, j:j + 1], axis=0),
                    )
                    pt = psA.tile([128, 128], f32, tag="pt", name=f"pt_{j}")
                    nc.tensor.transpose(
                        out=pt[:, :cnt], in_=ga[:cnt, :],
                        identity=ident[:cnt, :cnt])
                    for (k, t0, c0, w) in segs:
                        # split the segment at output-chunk boundaries
                        t = t0
                        while t < t0 + w:
                            c = min(t // 128, 3)
                            o_c = CHUNKS[c][0]
                            wp = min(t0 + w - t, CHUNKS[c][0] + CHUNKS[c][1] - t)
                            nc.vector.tensor_copy(
                                out=gT[k][c][:, t - o_c:t - o_c + wp],
                                in_=pt[:, c0 + (t - t0):c0 + (t - t0) + wp])
                            t += wp

                # feat + graph close their PSUM group early (no gather dep),
                # then get pre-transposed to token-major while gathers run
                po = psB.tile([128, T], f32, name="po")
                nc.tensor.matmul(out=po[:], lhsT=featw_r[:, 400:528],
                                 rhs=featw_r[:, 0:400],
                                 start=True, stop=False)
                nc.tensor.matmul(out=po[:], lhsT=wbig_r[:, 256:384],
                                 rhs=graphT_r[:, :].to_broadcast([128, BL, M]),
                                 start=False, stop=True)
                ctx_sb = sb.tile([128, T], f32, name="ctx_sb")
                nc.vector.tensor_copy(out=ctx_sb[:], in_=po[:])
                ctxT = []
                for c, (o, cnt) in enumerate(CHUNKS):
                    pt2 = psC.tile([128, 128], f32, tag="pt2", name=f"pt2_{c}")
                    nc.tensor.transpose(
                        out=pt2[:cnt, :], in_=ctx_sb[:, o:o + cnt],
                        identity=ident[:, :])
                    ctxT_c = sba.tile([128, 128], f32, tag=f"ctxT_{c}",
                                      name=f"ctxT_{c}")
                    nc.vector.tensor_copy(out=ctxT_c[:cnt, :], in_=pt2[:cnt, :])
                    ctxT.append(ctxT_c)

                # per-chunk gathered-embedding matmuls, token-major (the
                # gathered tile is lhsT), so each chunk's PSUM is already
                # in output layout: one DVE add then DMA out
                for c, (o, cnt) in enumerate(CHUNKS):
                    pg = psD.tile([128, 128], f32, tag="pg", name=f"pg_{c}")
                    nc.tensor.matmul(out=pg[:cnt, :], lhsT=gT[0][c][:, :],
                                     rhs=wbig_r[:, 0:128],
                                     start=True, stop=False)
                    nc.tensor.matmul(out=pg[:cnt, :], lhsT=gT[1][c][:, :],
                                     rhs=wbig_r[:, 128:256],
                                     start=False, stop=True)
                    ob = sbo.tile([128, E], f32, tag="ob", name=f"ob_{c}")
                    nc.vector.tensor_tensor(
                        out=ob[:cnt, :], in0=ctxT[c][:cnt, :],
                        in1=pg[:cnt, :], op=mybir.AluOpType.add)
                    eng = nc.sync if c % 2 == 0 else nc.scalar
                    eng.dma_start(out=out[o:o + cnt, :], in_=ob[:cnt, :])

    nc.compile()
    return nc, names


def _host_prep(inputs):
    """Fuse the linear layers (the module has no nonlinearity) and lay out
    per-core device inputs."""
    f64 = np.float64
    W_a = np.asarray(inputs["W_a"], f64)
    Wa0, Wa1 = W_a[:, :E], W_a[:, E:]
    W_dp = np.asarray(inputs["W_dp"], f64)
    Wf0 = Wa1 @ W_dp[:, :E]
    Wf1 = Wa1 @ W_dp[:, E:]
    Wfc = Wa1 @ np.asarray(inputs["W_dc"], f64)
    Wfn = Wa1 @ np.asarray(inputs["W_nc"], f64)
    Wfp = Wa1 @ np.asarray(inputs["W_ps"], f64)
    Wfg = Wa0 @ np.asarray(inputs["W_g"], f64)
    b_sum = (np.asarray(inputs["b_dp"], f64) + np.asarray(inputs["b_dc"], f64)
             + np.asarray(inputs["b_nc"], f64) + np.asarray(inputs["b_ps"], f64))
    b_total = (np.asarray(inputs["b_a"], f64) + Wa1 @ b_sum
               + Wa0 @ np.asarray(inputs["b_g"], f64))

    # featw: [12, 528] = [features+ones | (Wff_ext)^T]
    Wff_ext = np.concatenate(
        [Wfc, Wfn, Wfp, b_total[:, None]], axis=1)          # [128, 12]
    # wbig: [128, 384] = [Wf0^T | Wf1^T | Wfg^T]
    wbig = np.concatenate([Wf0.T, Wf1.T, Wfg.T], axis=1).astype(np.float32)
    wbig = np.ascontiguousarray(wbig)

    cities_embed = np.asarray(inputs["cities_embed"], np.float32)
    graph_embed = np.asarray(inputs["graph_embed"], np.float32)
    agent_state = np.asarray(inputs["agent_state"], np.float32)

    # packed gather grid: slot (p, j) is flat row r = j*128 + p, where
    # r = k*T + t; col j+8 carries the b*10000 offset constant
    p_grid = np.arange(128)[:, None]
    j_grid = np.arange(NG + 1)[None, :]          # 8 cols, col 7 unused pad
    r_grid = j_grid * 128 + p_grid
    valid = r_grid < R
    k_grid = np.where(valid, r_grid // T, 0)
    t_grid = np.where(valid, r_grid % T, 0)
    bofs = ((t_grid // M) * N * valid).astype(np.float32)

    in_maps = []
    for core in range(NCORES):
        bsl = slice(core * BL, (core + 1) * BL)
        ag = agent_state[bsl].reshape(T, 13)
        idx2 = np.zeros((128, 16), np.float32)
        idx2[:, 0:8] = ag[t_grid, k_grid] * valid
        idx2[:, 8:16] = bofs
        featw = np.zeros((12, 528), np.float32)
        featw[:11, :T] = ag[:, 2:13].T
        featw[11, :T] = 1.0
        featw[:, 400:528] = Wff_ext.T.astype(np.float32)
        in_maps.append({
            "cities": np.ascontiguousarray(cities_embed[bsl].reshape(BL * N, E)),
            "idx2": idx2,
            "featw": featw,
            "wbig": wbig,
            "graphT": np.ascontiguousarray(graph_embed[bsl, 0, :].T),
        })
    return in_maps


def kernel(**inputs):
    global last_exec_time_ns
    trace = os.environ.get("BASS_KERNEL_TRACE", "") == "1"
    if trace:
        _install_trace_shims()

    from concourse.bass_utils import run_bass_kernel_spmd

    if "nc" not in _cache:
        _cache["nc"], _cache["names"] = _build_nc()
    nc, names = _cache["nc"], _cache["names"]

    in_maps = []
    for m in _host_prep(inputs):
        in_maps.append({names[k]: v for k, v in m.items()})

    kwargs = {}
    if trace:
        tdir = os.environ.get("BASS_KERNEL_TRACE_DIR", "/tmp/kern_trace")
        import shutil
        shutil.rmtree(tdir, ignore_errors=True)
        os.makedirs(tdir, exist_ok=True)
        kwargs = dict(trace=True, trace_cores=list(range(NCORES)), tmpdir=tdir)
    res = run_bass_kernel_spmd(nc, in_maps, core_ids=list(range(NCORES)), **kwargs)
    last_exec_time_ns = res.exec_time_ns

    out = np.stack([res.results[i][names["out"]] for i in range(NCORES)])
    return out.reshape(B, M, E).astype(np.float32)


# revision 18
# speedup vs baseline: 1.1553x; 1.0117x over previous
"""Trainium2 Bass kernel for nn_AgentEmbedding (embedding_lookup).

Contract: kernel(**inputs) takes the FULL unsharded inputs (numpy arrays,
keyed as in setup_inputs()) and returns the FULL [64, 50, 128] float32
output. Internally the batch dim B=64 is sharded 8-ways (8 per core);
the small linear weights are algebraically fused on the host (the module
is linear end-to-end) and replicated.

Per-core device program (B_local=8, T=400 tokens, E=128):
  1. gather row ids = idx_float + b*10000, computed on DVE (int32)
  2. 7 indirect-DMA gathers (one row id per dest partition, 512B rows)
     pull the 2*T=800 needed rows out of the flat [80000, 128] table
  3. PE transposes gathered tiles to feature-major [E, tok]
  4. one fused linear: PSUM accumulation of 4 fp32r matmuls
     (features+bias via homogeneous ones-row, graph via broadcast rhs,
      two gathered-embedding terms)
  5. PE transposes back to token-major, DMA out
"""

import os
import numpy as np

B, M, N, E = 64, 50, 10000, 128
NCORES = 8
BL = B // NCORES            # batches per core
T = BL * M                  # tokens per core
R = 2 * T                   # gathered rows per core
NG = 7                      # ceil(800/128) gather instructions
CHUNKS = [(0, 128), (128, 128), (256, 128), (384, 16)]  # output chunks

_cache = {}

last_exec_time_ns = None


def _install_trace_shims():
    """antenv.axon_hooks is absent in this image; register the NTFF hook
    ourselves so run_bass_kernel_spmd(trace=True) works under axon."""
    import sys, types
    if "antenv.axon_hooks" not in sys.modules:
        mod = types.ModuleType("antenv.axon_hooks")
        store = {}
        mod.set_axon_ntff_profile_hook = lambda h: store.__setitem__("h", h)
        mod.get_axon_ntff_profile_hook = lambda: store.get("h")
        sys.modules["antenv.axon_hooks"] = mod
        try:
            from trn_agent_boot.trn_boot import _ntff_profile_via_ctypes
            mod.set_axon_ntff_profile_hook(
                _ntff_profile_via_ctypes("/opt/axon/libaxon_pjrt.so")
            )
        except Exception:
            pass
    import concourse.bass_utils as bu
    bu.upload_artifacts = lambda d: d  # zero-egress container


def _gather_plan():
    """For each gather chunk j: (rows, [(k, tok0, col0, width), ...])
    mapping transpose output columns to g{k}T column ranges."""
    plan = []
    for j in range(NG):
        r0 = j * 128
        cnt = min(128, R - r0)
        segs = []
        r = r0
        while r < r0 + cnt:
            k, t = divmod(r, T)
            width = min(r0 + cnt - r, T - t)
            segs.append((k, t, r - r0, width))
            r += width
        plan.append((cnt, segs))
    return plan


def _build_nc():
    """Build + compile the per-core Bass program (SPMD: same program on
    all 8 cores, per-core input data)."""
    import concourse.bass as bass
    import concourse.bacc as bacc
    import concourse.mybir as mybir
    import concourse.tile as tile
    from concourse.masks import make_identity

    f32 = mybir.dt.float32
    f32r = mybir.dt.float32r
    i32 = mybir.dt.int32

    nc = bacc.Bacc("TRN2", target_bir_lowering=False,
                   dynamic_dma_scratch_size=65536)
    with tile.TileContext(nc) as tc:
        with tc.tile_pool(name="dram", bufs=1, space="DRAM") as dram:
            cities = dram.tile([BL * N, E], f32, kind="ExternalInput", name="cities")
            idx2 = dram.tile([128, 16], f32, kind="ExternalInput", name="idx2")
            featw = dram.tile([12, 528], f32, kind="ExternalInput", name="featw")
            wbig = dram.tile([128, 384], f32, kind="ExternalInput", name="wbig")
            graphT = dram.tile([128, 8], f32, kind="ExternalInput", name="graphT")
            out = dram.tile([T, E], f32, kind="ExternalOutput", name="out")
            names = dict(cities=cities.name, idx2=idx2.name, featw=featw.name,
                         wbig=wbig.name, graphT=graphT.name, out=out.name)

            with (
                tc.tile_pool(name="sb", bufs=1) as sb,
                tc.tile_pool(name="sbg", bufs=4) as sbg,
                tc.tile_pool(name="sbo", bufs=4) as sbo,
                tc.tile_pool(name="sba", bufs=4) as sba,
                tc.tile_pool(name="psA", bufs=4, space="PSUM") as psA,
                tc.tile_pool(name="psB", bufs=1, space="PSUM") as psB,
                tc.tile_pool(name="psC", bufs=1, space="PSUM") as psC,
                tc.tile_pool(name="psD", bufs=2, space="PSUM") as psD,
            ):
                idx2_sb = sb.tile([128, 16], f32, name="idx2_sb")
                nc.sync.dma_start(out=idx2_sb[:], in_=idx2[:])
                graphT_sb = sb.tile([128, 8], f32, name="graphT_sb")
                nc.sync.dma_start(out=graphT_sb[:], in_=graphT[:])
                featw_sb = sb.tile([12, 528], f32, name="featw_sb")
                nc.scalar.dma_start(out=featw_sb[:], in_=featw[:])
                wbig_sb = sb.tile([128, 384], f32, name="wbig_sb")
                nc.scalar.dma_start(out=wbig_sb[:], in_=wbig[:])

                ident = sb.tile([128, 128], f32, name="ident")
                make_identity(nc, ident[:])

                # idx = idx_float + b*10000; exact-integer floats, so the
                # f32 -> i32 conversion is exact under any rounding mode
                idxi = sb.tile([128, 8], i32, name="idxi")
                nc.vector.tensor_tensor(
                    out=idxi[:], in0=idx2_sb[:, 0:8], in1=idx2_sb[:, 8:16],
                    op=mybir.AluOpType.add)

                # fp32r operands must be produced by a rounding instruction
                featw_r = sb.tile([12, 528], f32r, name="featw_r")
                nc.vector.tensor_copy(out=featw_r[:], in_=featw_sb[:])
                wbig_r = sb.tile([128, 384], f32r, name="wbig_r")
                nc.vector.tensor_copy(out=wbig_r[:], in_=wbig_sb[:])
                graphT_r = sb.tile([128, 8], f32r, name="graphT_r")
                nc.vector.tensor_copy(out=graphT_r[:], in_=graphT_sb[:])

                # separate per-(k, output-chunk) tiles so chunk matmuls
                # only depend on the gathers that feed them
                gT = [[sb.tile([128, cc], f32r, name=f"g{k}T_{c}")
                       for c, (oo, cc) in enumerate(CHUNKS)]
                      for k in (0, 1)]
                for j, (cnt, segs) in enumerate(_gather_plan()):
                    ga = sbg.tile([128, E], f32, tag="ga", name=f"ga_{j}")
                    nc.gpsimd.indirect_dma_start(
                        out=ga[:cnt, :],
                        out_offset=None,
                        in_=cities[:, :],
                        in_offset=bass.IndirectOffsetOnAxis(
                            ap=idxi[:cnt, j:j + 1], axis=0),
                    )
                    pt = psA.tile([128, 128], f32, tag="pt", name=f"pt_{j}")
                    nc.tensor.transpose(
                        out=pt[:, :cnt], in_=ga[:cnt, :],
                        identity=ident[:cnt, :cnt])
                    for (k, t0, c0, w) in segs:
                        # split the segment at output-chunk boundaries
                        t = t0
                        while t < t0 + w:
                            c = min(t // 128, 3)
                            o_c = CHUNKS[c][0]
                            wp = min(t0 + w - t, CHUNKS[c][0] + CHUNKS[c][1] - t)
                            nc.vector.tensor_copy(
                                out=gT[k][c][:, t - o_c:t - o_c + wp],
                                in_=pt[:, c0 + (t - t0):c0 + (t - t0) + wp])
                            t += wp

                # feat + graph close their PSUM group early (no gather dep),
                # then get pre-transposed to token-major while gathers run
                po = psB.tile([128, T], f32, name="po")
                nc.tensor.matmul(out=po[:], lhsT=featw_r[:, 400:528],
                                 rhs=featw_r[:, 0:400],
                                 start=True, stop=False)
                nc.tensor.matmul(out=po[:], lhsT=wbig_r[:, 256:384],
                                 rhs=graphT_r[:, :].to_broadcast([128, BL, M]),
                                 start=False, stop=True)
                ctx_sb = sb.tile([128, T], f32, name="ctx_sb")
                nc.vector.tensor_copy(out=ctx_sb[:], in_=po[:])
                ctxT = []
                for c, (o, cnt) in enumerate(CHUNKS):
                    pt2 = psC.tile([128, 128], f32, tag="pt2", name=f"pt2_{c}")
                    nc.tensor.transpose(
                        out=pt2[:cnt, :], in_=ctx_sb[:, o:o + cnt],
                        identity=ident[:, :])
                    ctxT_c = sba.tile([128, 128], f32, tag=f"ctxT_{c}",
                                      name=f"ctxT_{c}")
                    nc.vector.tensor_copy(out=ctxT_c[:cnt, :], in_=pt2[:cnt, :])
                    ctxT.append(ctxT_c)

                # per-chunk gathered-embedding matmuls, token-major (the
                # gathered tile is lhsT), so each chunk's PSUM is already
                # in output layout: one DVE add then DMA out
                for c, (o, cnt) in enumerate(CHUNKS):
                    pg = psD.tile([128, 128], f32, tag="pg", name=f"pg_{c}")
                    nc.tensor.matmul(out=pg[:cnt, :], lhsT=gT[0][c][:, :],
                                     rhs=wbig_r[:, 0:128],
                                     start=True, stop=False)
                    nc.tensor.matmul(out=pg[:cnt, :], lhsT=gT[1][c][:, :],
                                     rhs=wbig_r[:, 128:256],
                                     start=False, stop=True)
                    ob = sbo.tile([128, E], f32, tag="ob", name=f"ob_{c}")
                    nc.vector.tensor_tensor(
                        out=ob[:cnt, :], in0=ctxT[c][:cnt, :],
                        in1=pg[:cnt, :], op=mybir.AluOpType.add)
                    eng = nc.sync if c % 2 == 0 else nc.scalar
                    eng.dma_start(out=out[o:o + cnt, :], in_=ob[:cnt, :])

    nc.compile()
    return nc, names


def _host_prep(inputs):
    """Fuse the linear layers (the module has no nonlinearity) and lay out
    per-core device inputs."""
    f64 = np.float64
    W_a = np.asarray(inputs["W_a"], f64)
    Wa0, Wa1 = W_a[:, :E], W_a[:, E:]
    W_dp = np.asarray(inputs["W_dp"], f64)
    Wf0 = Wa1 @ W_dp[:, :E]
    Wf1 = Wa1 @ W_dp[:, E:]
    Wfc = Wa1 @ np.asarray(inputs["W_dc"], f64)
    Wfn = Wa1 @ np.asarray(inputs["W_nc"], f64)
    Wfp = Wa1 @ np.asarray(inputs["W_ps"], f64)
    Wfg = Wa0 @ np.asarray(inputs["W_g"], f64)
    b_sum = (np.asarray(inputs["b_dp"], f64) + np.asarray(inputs["b_dc"], f64)
             + np.asarray(inputs["b_nc"], f64) + np.asarray(inputs["b_ps"], f64))
    b_total = (np.asarray(inputs["b_a"], f64) + Wa1 @ b_sum
               + Wa0 @ np.asarray(inputs["b_g"], f64))

    # featw: [12, 528] = [features+ones | (Wff_ext)^T]
    Wff_ext = np.concatenate(
        [Wfc, Wfn, Wfp, b_total[:, None]], axis=1)          # [128, 12]
    # wbig: [128, 384] = [Wf0^T | Wf1^T | Wfg^T]
    wbig = np.concatenate([Wf0.T, Wf1.T, Wfg.T], axis=1).astype(np.float32)
    wbig = np.ascontiguousarray(wbig)

    cities_embed = np.asarray(inputs["cities_embed"], np.float32)
    graph_embed = np.asarray(inputs["graph_embed"], np.float32)
    agent_state = np.asarray(inputs["agent_state"], np.float32)

    # packed gather grid: slot (p, j) is flat row r = j*128 + p, where
    # r = k*T + t; col j+8 carries the b*10000 offset constant
    p_grid = np.arange(128)[:, None]
    j_grid = np.arange(NG + 1)[None, :]          # 8 cols, col 7 unused pad
    r_grid = j_grid * 128 + p_grid
    valid = r_grid < R
    k_grid = np.where(valid, r_grid // T, 0)
    t_grid = np.where(valid, r_grid % T, 0)
    bofs = ((t_grid // M) * N * valid).astype(np.float32)

    in_maps = []
    for core in range(NCORES):
        bsl = slice(core * BL, (core + 1) * BL)
        ag = agent_state[bsl].reshape(T, 13)
        idx2 = np.zeros((128, 16), np.float32)
        idx2[:, 0:8] = ag[t_grid, k_grid] * valid
        idx2[:, 8:16] = bofs
        featw = np.zeros((12, 528), np.float32)
        featw[:11, :T] = ag[:, 2:13].T
        featw[11, :T] = 1.0
        featw[:, 400:528] = Wff_ext.T.astype(np.float32)
        in_maps.append({
            "cities": np.ascontiguousarray(cities_embed[bsl].reshape(BL * N, E)),
            "idx2": idx2,
            "featw": featw,
            "wbig": wbig,
            "graphT": np.ascontiguousarray(graph_embed[bsl, 0, :].T),
        })
    return in_maps


def kernel(**inputs):
    global last_exec_time_ns
    trace = os.environ.get("BASS_KERNEL_TRACE", "") == "1"
    if trace:
        _install_trace_shims()

    from concourse.bass_utils import run_bass_kernel_spmd

    if "nc" not in _cache:
        _cache["nc"], _cache["names"] = _build_nc()
    nc, names = _cache["nc"], _cache["names"]

    in_maps = []
    for m in _host_prep(inputs):
        in_maps.append({names[k]: v for k, v in m.items()})

    kwargs = {}
    if trace:
        tdir = os.environ.get("BASS_KERNEL_TRACE_DIR", "/tmp/kern_trace")
        import shutil
        shutil.rmtree(tdir, ignore_errors=True)
        os.makedirs(tdir, exist_ok=True)
        kwargs = dict(trace=True, trace_cores=list(range(NCORES)), tmpdir=tdir)
    res = run_bass_kernel_spmd(nc, in_maps, core_ids=list(range(NCORES)), **kwargs)
    last_exec_time_ns = res.exec_time_ns

    out = np.stack([res.results[i][names["out"]] for i in range(NCORES)])
    return out.reshape(B, M, E).astype(np.float32)
